# revision 9
# baseline (speedup 1.0000x reference)
"""GCNDecoder on 8 Trainium2 NeuronCores (Bass/Tile).

Math (per layer, reordered to aggregate in the *input* feature dim — 2-3x
less gather traffic than aggregating the matmul output):

    T = A_hat @ H + diag(dinv^2) @ H          (message passing, width D_in)
    Z = T @ W (+ b3 on the last layer)
    stats = allreduce(colsum(Z), colsum(Z^2)); H' = relu(Z*scale + shift)
    allgather(H' shards) -> next layer's gather table

Sharding: data-parallel over nodes in a padded node space
(8 cores x BPC blocks x 128). Per destination block the kernel dma_gathers
the source rows of all incoming edges from the full (replicated) table and
segment-sums them on the TensorEngine with host-built coef-one-hot
selection matrices, accumulating in PSUM. dma_gather indices are int16, so
each block's edges are split into src<32768 ("lo") and src>=32768 ("hi")
gathers. The self-loop term is one extra diagonal selection tile whose rhs
is the core's own rows.
"""

import math
import sys

sys.path.insert(0, "/opt/trn_rl_repo")

import numpy as np
import ml_dtypes
from contextlib import ExitStack

import concourse.bass as bass
import concourse.bacc as bacc
import concourse.mybir as mybir
import concourse.tile as tile
from concourse import library_config
from concourse.masks import make_identity

BF16 = mybir.dt.bfloat16
F32 = mybir.dt.float32
I16 = mybir.dt.int16

P = 128
EPS = 1e-5


class CFG:
    def __init__(self, n_real, dims, n_cores=8, lo_lim=32768):
        self.NC = n_cores
        self.N_REAL = n_real
        self.NPC = n_real // n_cores            # real nodes per core
        assert self.NPC * n_cores == n_real
        self.BPC = math.ceil(self.NPC / P)      # blocks per core
        self.SH = self.BPC * P                  # padded shard rows
        self.NPAD = self.SH * n_cores
        self.IN, self.H1, self.H2, self.OUT = dims
        self.NBLK = self.BPC * n_cores          # global block count
        self.LO_LIM = lo_lim
        assert self.LO_LIM < self.NPAD


def _pad_id(n, cfg):
    return (n // cfg.NPC) * cfg.SH + (n % cfg.NPC)


def host_prep(x, edge_index, edge_attr, W1, b1, g1, be1, W2, b2, g2, be2, W3, b3,
              cfg: CFG):
    """All numpy preprocessing; returns (in_maps, TLO, THI)."""
    bf = ml_dtypes.bfloat16
    src = np.asarray(edge_index[0], dtype=np.int64)
    dst = np.asarray(edge_index[1], dtype=np.int64)
    w = np.asarray(edge_attr, dtype=np.float32)[:, 2]
    E = src.shape[0]

    deg = np.bincount(dst, weights=w, minlength=cfg.N_REAL).astype(np.float32) + 1.0
    dinv = (1.0 / np.sqrt(deg)).astype(np.float32)
    coef = (dinv[src] * w * dinv[dst]).astype(np.float32)

    psrc = _pad_id(src, cfg)
    pdst = _pad_id(dst, cfg)
    gb = pdst // P                       # global dst block
    grp = (psrc >= cfg.LO_LIM).astype(np.int64)

    order = np.lexsort((psrc, grp, gb))
    gb_s = gb[order]
    grp_s = grp[order]
    src_s = psrc[order]
    dstl_s = (pdst[order] % P).astype(np.int64)
    coef_s = coef[order]

    key = gb_s * 2 + grp_s
    cnt = np.bincount(key, minlength=cfg.NBLK * 2)
    starts = np.concatenate([[0], np.cumsum(cnt)])
    rank = np.arange(E) - starts[key]
    TLO = max(1, math.ceil(cnt[0::2].max() / P))
    THI = max(1, math.ceil(cnt[1::2].max() / P))
    NT = TLO + THI + 1                    # +1 self tile

    slot = rank + np.where(grp_s == 1, TLO * P, 0)
    t_idx = slot // P
    p_idx = slot % P
    core_s = gb_s // cfg.BPC
    b_s = gb_s % cfg.BPC

    # selection matrices [core, block, tile, edge_part, dst_local]
    S = np.zeros((cfg.NC, cfg.BPC, NT, P, P), dtype=bf)
    S[core_s, b_s, t_idx, p_idx, dstl_s] = coef_s.astype(bf)
    dinv2_pad = np.zeros(cfg.NPAD, np.float32)
    dinv2_pad[_pad_id(np.arange(cfg.N_REAL), cfg)] = dinv * dinv
    diag = dinv2_pad.reshape(cfg.NC, cfg.BPC, P)
    pr = np.arange(P)
    S[:, :, NT - 1, pr, pr] = diag.astype(bf)
    # SBUF layout [128, BPC*NT*128]
    S_dram = np.ascontiguousarray(
        np.transpose(S, (0, 3, 1, 2, 4)).reshape(cfg.NC, P, cfg.BPC * NT * P))

    def wrap_idx(ids):  # [NC, BPC, T*128] -> [NC, 128, BPC*T*8]
        nc_, bpc_, n_ = ids.shape
        wr = ids.reshape(nc_, bpc_, n_ // 16, 16)
        wr = np.transpose(wr, (0, 3, 1, 2))                   # [nc,16,bpc,s]
        wr = np.tile(wr, (1, 8, 1, 1))                        # [nc,128,bpc,s]
        return np.ascontiguousarray(wr.reshape(nc_, P, bpc_ * (n_ // 16)))

    idx_lo = np.zeros((cfg.NC, cfg.BPC, TLO * P), np.int16)
    idx_hi = np.zeros((cfg.NC, cfg.BPC, THI * P), np.int16)
    m_lo = grp_s == 0
    m_hi = ~m_lo
    idx_lo[core_s[m_lo], b_s[m_lo], rank[m_lo]] = src_s[m_lo].astype(np.int16)
    idx_hi[core_s[m_hi], b_s[m_hi], rank[m_hi]] = (
        src_s[m_hi] - cfg.LO_LIM).astype(np.int16)
    idx_lo_d = wrap_idx(idx_lo)
    idx_hi_d = wrap_idx(idx_hi)

    x_pad = np.zeros((cfg.NPAD, cfg.IN), dtype=bf)
    x_pad[_pad_id(np.arange(cfg.N_REAL), cfg)] = np.asarray(x, np.float32).astype(bf)

    b3b = np.tile(np.asarray(b3, np.float32).reshape(1, cfg.OUT), (P, 1))

    in_maps = []
    for c in range(cfg.NC):
        in_maps.append({
            "x_full": x_pad,
            "x_own": np.ascontiguousarray(x_pad[c * cfg.SH:(c + 1) * cfg.SH]),
            "s_tab": S_dram[c],
            "idx_lo": idx_lo_d[c],
            "idx_hi": idx_hi_d[c],
            "w1": np.asarray(W1, np.float32).astype(bf),
            "w2": np.asarray(W2, np.float32).astype(bf),
            "w3": np.asarray(W3, np.float32).astype(bf),
            "g1": np.asarray(g1, np.float32).reshape(1, -1),
            "be1": np.asarray(be1, np.float32).reshape(1, -1),
            "g2": np.asarray(g2, np.float32).reshape(1, -1),
            "be2": np.asarray(be2, np.float32).reshape(1, -1),
            "b3b": b3b.astype(np.float32),
        })
    return in_maps, TLO, THI


def build_program(cfg: CFG, TLO, THI):
    NT = TLO + THI + 1
    nc = bacc.Bacc("TRN2", target_bir_lowering=False, debug=False,
                   num_devices=cfg.NC)

    x_full = nc.dram_tensor("x_full", [cfg.NPAD, cfg.IN], BF16, kind="ExternalInput")
    x_own = nc.dram_tensor("x_own", [cfg.SH, cfg.IN], BF16, kind="ExternalInput")
    s_tab = nc.dram_tensor("s_tab", [P, cfg.BPC * NT * P], BF16, kind="ExternalInput")
    idx_lo_d = nc.dram_tensor("idx_lo", [P, cfg.BPC * TLO * 8], I16,
                              kind="ExternalInput")
    idx_hi_d = nc.dram_tensor("idx_hi", [P, cfg.BPC * THI * 8], I16,
                              kind="ExternalInput")
    w_d = {
        1: nc.dram_tensor("w1", [cfg.IN, cfg.H1], BF16, kind="ExternalInput"),
        2: nc.dram_tensor("w2", [cfg.H1, cfg.H2], BF16, kind="ExternalInput"),
        3: nc.dram_tensor("w3", [cfg.H2, cfg.OUT], BF16, kind="ExternalInput"),
    }
    g_d = {1: nc.dram_tensor("g1", [1, cfg.H1], F32, kind="ExternalInput"),
           2: nc.dram_tensor("g2", [1, cfg.H2], F32, kind="ExternalInput")}
    be_d = {1: nc.dram_tensor("be1", [1, cfg.H1], F32, kind="ExternalInput"),
            2: nc.dram_tensor("be2", [1, cfg.H2], F32, kind="ExternalInput")}
    b3b_d = nc.dram_tensor("b3b", [P, cfg.OUT], F32, kind="ExternalInput")
    out_d = nc.dram_tensor("out", [cfg.SH, cfg.OUT], F32, kind="ExternalOutput")

    rg = [list(range(cfg.NC))]

    with tile.TileContext(nc, num_cores=cfg.NC) as tc, ExitStack() as top:
        nc.gpsimd.load_library(library_config.mlp)

        dram = top.enter_context(tc.tile_pool(name="dram", bufs=1, space="DRAM"))
        h_own = {l: dram.tile([cfg.SH, d], BF16, tag=f"h{l}o", name=f"h{l}o")
                 for l, d in ((1, cfg.H1), (2, cfg.H2))}
        h_full = {l: dram.tile([cfg.NPAD, d], BF16, tag=f"h{l}f", name=f"h{l}f",
                               addr_space="Shared")
                  for l, d in ((1, cfg.H1), (2, cfg.H2))}
        st_in = {l: dram.tile([1, 2 * d], F32, tag=f"sti{l}", name=f"sti{l}")
                 for l, d in ((1, cfg.H1), (2, cfg.H2))}
        st_out = {l: dram.tile([1, 2 * d], F32, tag=f"sto{l}", name=f"sto{l}",
                               addr_space="Shared")
                  for l, d in ((1, cfg.H1), (2, cfg.H2))}
        ss_dr = {l: dram.tile([1, 2 * d], F32, tag=f"ssd{l}", name=f"ssd{l}")
                 for l, d in ((1, cfg.H1), (2, cfg.H2))}
        z_sp = {l: dram.tile([cfg.SH, d], BF16, tag=f"zsp{l}", name=f"zsp{l}")
                for l, d in ((1, cfg.H1), (2, cfg.H2))}

        cpool = top.enter_context(tc.tile_pool(name="const", bufs=1))
        ones_t = cpool.tile([P, 1], BF16, tag="ones")
        nc.vector.memset(ones_t[:], 1.0)
        ident = cpool.tile([P, P], BF16, tag="ident")
        make_identity(nc, ident[:])
        idx_lo_t = cpool.tile([P, cfg.BPC * TLO * 8], I16, tag="idxlo")
        nc.sync.dma_start(idx_lo_t[:], idx_lo_d[:])
        idx_hi_t = cpool.tile([P, cfg.BPC * THI * 8], I16, tag="idxhi")
        nc.sync.dma_start(idx_hi_t[:], idx_hi_d[:])

        layers = [
            (1, cfg.IN, cfg.H1, x_full, x_own, True),
            (2, cfg.H1, cfg.H2, h_full[1], h_own[1], True),
            (3, cfg.H2, cfg.OUT, h_full[2], h_own[2], False),
        ]

        for (li, DI, DO, tab, own, bn) in layers:
            KI = DI // P
            n_oc = math.ceil(DO / 512)
            with ExitStack() as ls:
                sb = ls.enter_context(tc.tile_pool(name=f"sb{li}", bufs=2))
                sb1 = ls.enter_context(tc.tile_pool(name=f"sbw{li}", bufs=1))
                psA = ls.enter_context(tc.tile_pool(name=f"psA{li}", bufs=2,
                                                    space="PSUM"))
                psB = ls.enter_context(tc.tile_pool(
                    name=f"psB{li}", bufs=(1 if bn else 2), space="PSUM"))
                if bn:
                    psS = ls.enter_context(tc.tile_pool(name=f"psS{li}", bufs=1,
                                                        space="PSUM"))
                    sum_ps = psS.tile([1, DO], F32, tag="sum", space="PSUM")
                    sq_ps = psS.tile([1, DO], F32, tag="sq", space="PSUM")

                w_t = sb1.tile([P, KI * DO], BF16, tag="w")
                for k in range(KI):
                    nc.sync.dma_start(w_t[:, k * DO:(k + 1) * DO],
                                      w_d[li][k * P:(k + 1) * P, :])
                if not bn:
                    b3_t = sb1.tile([P, cfg.OUT], F32, tag="b3")
                    nc.sync.dma_start(b3_t[:], b3b_d[:])

                for b in range(cfg.BPC):
                    gl = sb.tile([P, TLO, DI], BF16, tag="gl")
                    nc.gpsimd.dma_gather(
                        gl[:], tab[0:cfg.LO_LIM, :],
                        idx_lo_t[:, b * TLO * 8:(b + 1) * TLO * 8],
                        TLO * P, TLO * P, DI,
                        single_packet=(TLO * P <= 1024))
                    gh = sb.tile([P, THI, DI], BF16, tag="gh")
                    nc.gpsimd.dma_gather(
                        gh[:], tab[cfg.LO_LIM:cfg.NPAD, :],
                        idx_hi_t[:, b * THI * 8:(b + 1) * THI * 8],
                        THI * P, THI * P, DI,
                        single_packet=(THI * P <= 1024))
                    own_t = sb.tile([P, DI], BF16, tag="own")
                    nc.scalar.dma_start(own_t[:], own[b * P:(b + 1) * P, :])
                    s_t = sb.tile([P, NT * P], BF16, tag="s")
                    nc.sync.dma_start(
                        s_t[:], s_tab[:, b * NT * P:(b + 1) * NT * P])

                    agg = psA.tile([P, DI], F32, tag="agg", space="PSUM")
                    for t in range(NT):
                        if t < TLO:
                            rhs = gl[:, t, :]
                        elif t < TLO + THI:
                            rhs = gh[:, t - TLO, :]
                        else:
                            rhs = own_t[:]
                        for f0 in range(0, DI, 512):
                            fw = min(512, DI - f0)
                            nc.tensor.matmul(
                                agg[:, f0:f0 + fw],
                                lhsT=s_t[:, t * P:(t + 1) * P],
                                rhs=rhs[:, f0:f0 + fw],
                                start=(t == 0), stop=(t == NT - 1))
                    T_t = sb.tile([P, DI], BF16, tag="T")
                    nc.vector.tensor_copy(T_t[:], agg[:])

                    TT_t = sb.tile([P, DI], BF16, tag="TT")
                    for k in range(KI):
                        tp = psB.tile([P, P], BF16, tag="tp", space="PSUM")
                        nc.tensor.transpose(tp[:], T_t[:, k * P:(k + 1) * P],
                                            ident[:])
                        nc.vector.tensor_copy(TT_t[:, k * P:(k + 1) * P], tp[:])

                    if bn:
                        z_t = sb.tile([P, DO], BF16, tag="z")
                    else:
                        o_t = sb.tile([P, cfg.OUT], F32, tag="o")
                    for oc in range(n_oc):
                        o0 = oc * 512
                        ow = min(512, DO - o0)
                        zp = psB.tile([P, ow], F32, tag="zp", space="PSUM")
                        for k in range(KI):
                            nc.tensor.matmul(
                                zp[:],
                                lhsT=TT_t[:, k * P:(k + 1) * P],
                                rhs=w_t[:, k * DO + o0:k * DO + o0 + ow],
                                start=(k == 0), stop=(k == KI - 1))
                        if bn:
                            nc.vector.tensor_copy(z_t[:, o0:o0 + ow], zp[:])
                        else:
                            nc.vector.tensor_add(o_t[:, o0:o0 + ow], zp[:],
                                                 b3_t[:, o0:o0 + ow])
                    if bn:
                        zsq = sb.tile([P, DO], BF16, tag="zsq")
                        nc.vector.tensor_tensor(
                            out=zsq[:], in0=z_t[:], in1=z_t[:],
                            op=mybir.AluOpType.mult)
                        for o0 in range(0, DO, 512):
                            ow = min(512, DO - o0)
                            nc.tensor.matmul(
                                sum_ps[:, o0:o0 + ow], lhsT=ones_t[:],
                                rhs=z_t[:, o0:o0 + ow],
                                start=(b == 0), stop=(b == cfg.BPC - 1))
                            nc.tensor.matmul(
                                sq_ps[:, o0:o0 + ow], lhsT=ones_t[:],
                                rhs=zsq[:, o0:o0 + ow],
                                start=(b == 0), stop=(b == cfg.BPC - 1))
                        nc.scalar.dma_start(z_sp[li][b * P:(b + 1) * P, :], z_t[:])
                    else:
                        nc.scalar.dma_start(out_d[b * P:(b + 1) * P, :], o_t[:])

                if bn:
                    st_l = sb1.tile([1, 2 * DO], F32, tag="stl")
                    nc.vector.tensor_copy(st_l[:, 0:DO], sum_ps[:])
                    nc.vector.tensor_copy(st_l[:, DO:2 * DO], sq_ps[:])
                    nc.sync.dma_start(st_in[li][:, :], st_l[:])
                    nc.gpsimd.collective_compute(
                        "AllReduce", mybir.AluOpType.add,
                        replica_groups=rg,
                        ins=[st_in[li].opt()],
                        outs=[st_out[li].opt()])
                    st_g = sb1.tile([1, 2 * DO], F32, tag="stg")
                    nc.sync.dma_start(st_g[:], st_out[li][:, :])
                    g_t = sb1.tile([1, DO], F32, tag="g")
                    nc.sync.dma_start(g_t[:], g_d[li][:, :])
                    be_t = sb1.tile([1, DO], F32, tag="be")
                    nc.sync.dma_start(be_t[:], be_d[li][:, :])

                    inv_n = 1.0 / cfg.N_REAL
                    mean = sb1.tile([1, DO], F32, tag="mean")
                    nc.vector.tensor_scalar_mul(mean[:], st_g[:, 0:DO], inv_n)
                    msq = sb1.tile([1, DO], F32, tag="msq")
                    nc.vector.tensor_scalar_mul(msq[:], st_g[:, DO:2 * DO], inv_n)
                    var = sb1.tile([1, DO], F32, tag="var")
                    nc.vector.tensor_tensor(out=var[:], in0=mean[:], in1=mean[:],
                                            op=mybir.AluOpType.mult)
                    nc.vector.tensor_sub(var[:], msq[:], var[:])
                    nc.vector.tensor_scalar_add(var[:], var[:], EPS)
                    rec = sb1.tile([1, DO], F32, tag="rec")
                    nc.vector.reciprocal(rec[:], var[:])
                    zb = sb1.tile([1, 1], F32, tag="zb")
                    nc.vector.memset(zb[:], 0.0)
                    rstd = sb1.tile([1, DO], F32, tag="rstd")
                    nc.scalar.activation(rstd[:], rec[:],
                                         mybir.ActivationFunctionType.Sqrt,
                                         bias=zb[:])
                    ss_l = sb1.tile([1, 2 * DO], F32, tag="ssl")
                    nc.vector.tensor_tensor(out=ss_l[:, 0:DO], in0=g_t[:],
                                            in1=rstd[:], op=mybir.AluOpType.mult)
                    tmp = sb1.tile([1, DO], F32, tag="tmp")
                    nc.vector.tensor_tensor(out=tmp[:], in0=mean[:],
                                            in1=ss_l[:, 0:DO],
                                            op=mybir.AluOpType.mult)
                    nc.vector.tensor_sub(ss_l[:, DO:2 * DO], be_t[:], tmp[:])
                    nc.sync.dma_start(ss_dr[li][:, :], ss_l[:])
                    ssb_f = sb1.tile([P, 2 * DO], F32, tag="ssbf")
                    nc.sync.dma_start(
                        ssb_f[:], ss_dr[li][0:1, :].to_broadcast([P, 2 * DO]))
                    ssb = sb1.tile([P, 2 * DO], BF16, tag="ssb")
                    nc.vector.tensor_copy(ssb[:], ssb_f[:])

                    for b in range(cfg.BPC):
                        zr = sb.tile([P, DO], BF16, tag="zr")
                        nc.sync.dma_start(zr[:], z_sp[li][b * P:(b + 1) * P, :])
                        h_t = sb.tile([P, DO], BF16, tag="h")
                        nc.vector.tensor_tensor(out=h_t[:], in0=zr[:],
                                                in1=ssb[:, 0:DO],
                                                op=mybir.AluOpType.mult)
                        nc.vector.tensor_tensor(out=h_t[:], in0=h_t[:],
                                                in1=ssb[:, DO:2 * DO],
                                                op=mybir.AluOpType.add)
                        nc.vector.tensor_relu(h_t[:], h_t[:])
                        nc.scalar.dma_start(
                            h_own[li][b * P:(b + 1) * P, :], h_t[:])
                    nc.gpsimd.collective_compute(
                        "AllGather", mybir.AluOpType.bypass,
                        replica_groups=rg,
                        ins=[h_own[li].opt()],
                        outs=[h_full[li].opt()])

    nc.compile()
    return nc


# ---------------------------------------------------------------------------

_FULL_CFG = CFG(50000, (256, 512, 1024, 3000))
_prog_cache = {}


def _get_program(cfg, TLO, THI):
    key = (cfg.N_REAL, cfg.IN, TLO, THI)
    if key not in _prog_cache:
        _prog_cache[key] = build_program(cfg, TLO, THI)
    return _prog_cache[key]


def kernel(x, edge_index, edge_attr, y, W1, b1, g1, be1, W2, b2, g2, be2, W3, b3):
    cfg = _FULL_CFG
    in_maps, TLO, THI = host_prep(
        x, edge_index, edge_attr, W1, b1, g1, be1, W2, b2, g2, be2, W3, b3, cfg)
    nc = _get_program(cfg, TLO, THI)

    from concourse.bass_utils import run_bass_kernel_spmd
    res = run_bass_kernel_spmd(nc, in_maps, core_ids=list(range(cfg.NC)))
    out = np.concatenate(
        [np.asarray(res.results[c]["out"])[:cfg.NPC] for c in range(cfg.NC)], axis=0)
    return out.astype(np.float32)


# revision 14
# speedup vs baseline: 1.0582x; 1.0582x over previous
"""GCNDecoder on 8 Trainium2 NeuronCores (Bass/Tile).

Math (per layer, reordered to aggregate in the *input* feature dim — 2-3x
less gather traffic than aggregating the matmul output):

    T = A_hat @ H + diag(dinv^2) @ H          (message passing, width D_in)
    Z = T @ W (+ b3 on the last layer)
    stats = allreduce(colsum(Z), colsum(Z^2)); H' = relu(Z*scale + shift)
    allgather(H' shards) -> next layer's gather table

Sharding: data-parallel over nodes in a padded node space laid out
chunk-major (chunk, core, block-in-chunk, 128) so the per-layer h
all-gather can run as several small chunk AllGathers (Mesh algorithm)
overlapped with the BatchNorm apply phase.

Per destination block the kernel dma_gathers the source rows of incoming
edges from the full (replicated) table and segment-sums them on the
TensorEngine with host-built coef-one-hot selection matrices accumulated in
PSUM. dma_gather descriptor generation on the Q7 costs ~8ns/row and is the
kernel's main floor, so the host dedups (block, src) pairs (a gathered row
feeds multiple one-hot columns) and uses per-block tile counts instead of a
global max. dma_gather indices are int16, so each block's edges split into
src<32768 ("lo") and src>=32768 ("hi") gathers. The self-loop term is one
extra diagonal selection tile whose rhs is the core's own rows (sequential
DMA, no descriptors).
"""

import math
import sys

sys.path.insert(0, "/opt/trn_rl_repo")

import numpy as np
import ml_dtypes
from contextlib import ExitStack

import concourse.bass as bass
import concourse.bacc as bacc
import concourse.mybir as mybir
import concourse.tile as tile
from concourse import library_config
from concourse.masks import make_identity

BF16 = mybir.dt.bfloat16
F32 = mybir.dt.float32
I16 = mybir.dt.int16

P = 128
EPS = 1e-5


class CFG:
    def __init__(self, n_real, dims, n_cores=8, lo_lim=32768, cg=7):
        self.NC = n_cores
        self.N_REAL = n_real
        self.NPC = n_real // n_cores            # real nodes per core
        assert self.NPC * n_cores == n_real
        self.BPC = math.ceil(self.NPC / P)      # blocks per core
        self.CG = cg                            # blocks per AG chunk
        self.SH = self.BPC * P                  # padded shard rows
        self.NPAD = self.SH * n_cores
        self.IN, self.H1, self.H2, self.OUT = dims
        self.NBLK = self.BPC * n_cores
        self.LO_LIM = lo_lim
        assert self.LO_LIM < self.NPAD
        assert self.BPC % cg == 0


def _pad_id(n, cfg):
    """node id -> padded id, chunk-major global layout:
    pid = ((g*NC + core)*CG + bc)*128 + r   (g = chunk, bc = block in chunk)"""
    core = n // cfg.NPC
    pos = n % cfg.NPC
    b = pos // P
    r = pos % P
    g = b // cfg.CG
    bc = b % cfg.CG
    return ((g * cfg.NC + core) * cfg.CG + bc) * P + r


def host_prep(x, edge_index, edge_attr, W1, b1, g1, be1, W2, b2, g2, be2, W3, b3,
              cfg: CFG):
    """All numpy preprocessing; returns (in_maps, tlos, this_)."""
    bf = ml_dtypes.bfloat16
    src = np.asarray(edge_index[0], dtype=np.int64)
    dst = np.asarray(edge_index[1], dtype=np.int64)
    w = np.asarray(edge_attr, dtype=np.float32)[:, 2]

    deg = np.bincount(dst, weights=w, minlength=cfg.N_REAL).astype(np.float32) + 1.0
    dinv = (1.0 / np.sqrt(deg)).astype(np.float32)
    coef = (dinv[src] * w * dinv[dst]).astype(np.float32)

    psrc = _pad_id(src, cfg)
    pdst = _pad_id(dst, cfg)
    core_e = (pdst // (cfg.CG * P)) % cfg.NC
    g_e = pdst // (cfg.NC * cfg.CG * P)
    bc_e = (pdst // P) % cfg.CG
    b_e = g_e * cfg.CG + bc_e                 # block index within core [0,BPC)
    dstl = pdst % P
    grp = (psrc >= cfg.LO_LIM).astype(np.int64)

    # dedup key: (core, block, grp, src)
    okey = (((core_e * cfg.BPC) + b_e) * 2 + grp) * cfg.NPAD + psrc
    order = np.argsort(okey, kind="stable")
    okey_s = okey[order]
    new_u = np.empty(len(order), bool)
    new_u[0] = True
    new_u[1:] = okey_s[1:] != okey_s[:-1]
    uid = np.cumsum(new_u) - 1                # unique (cb,grp,src) id per edge
    ugrp_key = okey_s[new_u] // cfg.NPAD      # (core*BPC+b)*2+grp per unique
    n_u = int(uid[-1]) + 1
    grp_start = np.searchsorted(ugrp_key, np.arange(cfg.NBLK * 2))
    u_rank = np.arange(n_u) - grp_start[ugrp_key]
    cnt_u = np.bincount(ugrp_key, minlength=cfg.NBLK * 2)

    # per-block-index tile counts: max over cores at the same block index
    cnt_lo = cnt_u[0::2].reshape(cfg.NC, cfg.BPC)
    cnt_hi = cnt_u[1::2].reshape(cfg.NC, cfg.BPC)
    tlos = [max(1, math.ceil(cnt_lo[:, j].max() / P)) for j in range(cfg.BPC)]
    this_ = [max(1, math.ceil(cnt_hi[:, j].max() / P)) for j in range(cfg.BPC)]
    nts = [tlos[j] + this_[j] + 1 for j in range(cfg.BPC)]

    off_lo = np.concatenate([[0], np.cumsum(tlos)]).astype(np.int64)   # tiles
    off_hi = np.concatenate([[0], np.cumsum(this_)]).astype(np.int64)
    off_s = np.concatenate([[0], np.cumsum(nts)]).astype(np.int64)
    TOT_LO, TOT_HI, TOT_S = int(off_lo[-1]), int(off_hi[-1]), int(off_s[-1])

    u_core = ugrp_key // (2 * cfg.BPC)
    u_b = (ugrp_key // 2) % cfg.BPC
    u_grp = ugrp_key % 2
    u_src = okey_s[new_u] % cfg.NPAD

    idx_lo = np.zeros((cfg.NC, TOT_LO * P), np.int16)
    idx_hi = np.zeros((cfg.NC, TOT_HI * P), np.int16)
    mlo = u_grp == 0
    mhi = ~mlo
    idx_lo[u_core[mlo], off_lo[u_b[mlo]] * P + u_rank[mlo]] = \
        u_src[mlo].astype(np.int16)
    idx_hi[u_core[mhi], off_hi[u_b[mhi]] * P + u_rank[mhi]] = \
        (u_src[mhi] - cfg.LO_LIM).astype(np.int16)

    # S: per-core [TOT_S tiles, 128, 128]; edge -> (tile, part) via its
    # unique's slot; column = dst_local; += coef (handles duplicates).
    S = np.zeros((cfg.NC, TOT_S, P, P), np.float32)
    tlos_a = np.asarray(tlos)
    e_core = u_core[uid]
    e_b = u_b[uid]
    e_rank = u_rank[uid]
    e_grp = u_grp[uid]
    slot = e_rank + np.where(e_grp == 1, tlos_a[e_b] * P, 0)
    tile_g = off_s[e_b] + slot // P
    part = slot % P
    np.add.at(S, (e_core, tile_g, part, dstl[order]), coef[order])
    # self tile: last tile of each block, diagonal = dinv2 (0 on pad rows)
    dinv2_pad = np.zeros(cfg.NPAD, np.float32)
    dinv2_pad[_pad_id(np.arange(cfg.N_REAL), cfg)] = dinv * dinv
    pr = np.arange(P)
    for j in range(cfg.BPC):
        st = off_s[j + 1] - 1
        for c in range(cfg.NC):
            g = j // cfg.CG
            bc = j % cfg.CG
            pid0 = ((g * cfg.NC + c) * cfg.CG + bc) * P
            S[c, st, pr, pr] = dinv2_pad[pid0 + pr]
    S_bf = S.astype(bf)
    S_dram = np.ascontiguousarray(
        np.transpose(S_bf, (0, 2, 1, 3)).reshape(cfg.NC, P, TOT_S * P))

    def wrap_idx(ids):  # [NC, T*128] -> [NC, 128, T*8] (per-16 wrap, x8)
        nc_, n_ = ids.shape
        wr = ids.reshape(nc_, n_ // 16, 16)
        wr = np.transpose(wr, (0, 2, 1))
        wr = np.tile(wr, (1, 8, 1))
        return np.ascontiguousarray(wr)

    idx_lo_d = wrap_idx(idx_lo)
    idx_hi_d = wrap_idx(idx_hi)

    x_pad = np.zeros((cfg.NPAD, cfg.IN), dtype=bf)
    x_pad[_pad_id(np.arange(cfg.N_REAL), cfg)] = \
        np.asarray(x, np.float32).astype(bf)

    # x_own: core-local b-major rows (own pids are strided in the global
    # chunk-major layout)
    j = np.arange(cfg.SH)
    bj = j // P
    rj = j % P
    gj = bj // cfg.CG
    bcj = bj % cfg.CG

    b3b = np.tile(np.asarray(b3, np.float32).reshape(1, cfg.OUT), (P, 1))

    in_maps = []
    for c in range(cfg.NC):
        own_pid = ((gj * cfg.NC + c) * cfg.CG + bcj) * P + rj
        in_maps.append({
            "x_full": x_pad,
            "x_own": np.ascontiguousarray(x_pad[own_pid]),
            "s_tab": S_dram[c],
            "idx_lo": idx_lo_d[c],
            "idx_hi": idx_hi_d[c],
            "w1": np.asarray(W1, np.float32).astype(bf),
            "w2": np.asarray(W2, np.float32).astype(bf),
            "w3": np.asarray(W3, np.float32).astype(bf),
            "g1": np.asarray(g1, np.float32).reshape(1, -1),
            "be1": np.asarray(be1, np.float32).reshape(1, -1),
            "g2": np.asarray(g2, np.float32).reshape(1, -1),
            "be2": np.asarray(be2, np.float32).reshape(1, -1),
            "b3b": b3b.astype(np.float32),
        })
    return in_maps, tlos, this_


def build_program(cfg: CFG, tlos, this_):
    nts = [tlos[j] + this_[j] + 1 for j in range(cfg.BPC)]
    off_lo = [0]
    off_hi = [0]
    off_s = [0]
    for j in range(cfg.BPC):
        off_lo.append(off_lo[-1] + tlos[j])
        off_hi.append(off_hi[-1] + this_[j])
        off_s.append(off_s[-1] + nts[j])
    TOT_LO, TOT_HI, TOT_S = off_lo[-1], off_hi[-1], off_s[-1]
    MAXLO, MAXHI, MAXNT = max(tlos), max(this_), max(nts)

    nc = bacc.Bacc("TRN2", target_bir_lowering=False, debug=False,
                   num_devices=cfg.NC)

    x_full = nc.dram_tensor("x_full", [cfg.NPAD, cfg.IN], BF16, kind="ExternalInput")
    x_own = nc.dram_tensor("x_own", [cfg.SH, cfg.IN], BF16, kind="ExternalInput")
    s_tab = nc.dram_tensor("s_tab", [P, TOT_S * P], BF16, kind="ExternalInput")
    idx_lo_d = nc.dram_tensor("idx_lo", [P, TOT_LO * 8], I16, kind="ExternalInput")
    idx_hi_d = nc.dram_tensor("idx_hi", [P, TOT_HI * 8], I16, kind="ExternalInput")
    w_d = {
        1: nc.dram_tensor("w1", [cfg.IN, cfg.H1], BF16, kind="ExternalInput"),
        2: nc.dram_tensor("w2", [cfg.H1, cfg.H2], BF16, kind="ExternalInput"),
        3: nc.dram_tensor("w3", [cfg.H2, cfg.OUT], BF16, kind="ExternalInput"),
    }
    g_d = {1: nc.dram_tensor("g1", [1, cfg.H1], F32, kind="ExternalInput"),
           2: nc.dram_tensor("g2", [1, cfg.H2], F32, kind="ExternalInput")}
    be_d = {1: nc.dram_tensor("be1", [1, cfg.H1], F32, kind="ExternalInput"),
            2: nc.dram_tensor("be2", [1, cfg.H2], F32, kind="ExternalInput")}
    b3b_d = nc.dram_tensor("b3b", [P, cfg.OUT], F32, kind="ExternalInput")
    out_d = nc.dram_tensor("out", [cfg.SH, cfg.OUT], BF16, kind="ExternalOutput")

    rg = [list(range(cfg.NC))]
    CROWS = cfg.CG * P                       # rows per (chunk, core)
    NG = cfg.BPC // cfg.CG                   # number of AG chunks

    with tile.TileContext(nc, num_cores=cfg.NC) as tc, ExitStack() as top:
        nc.gpsimd.load_library(library_config.mlp)

        dram = top.enter_context(tc.tile_pool(name="dram", bufs=1, space="DRAM"))
        h_own = {l: dram.tile([cfg.SH, d], BF16, tag=f"h{l}o", name=f"h{l}o")
                 for l, d in ((1, cfg.H1), (2, cfg.H2))}
        h_full = {l: dram.tile([cfg.NPAD, d], BF16, tag=f"h{l}f", name=f"h{l}f")
                  for l, d in ((1, cfg.H1), (2, cfg.H2))}
        st_in = {l: dram.tile([1, 2 * d], F32, tag=f"sti{l}", name=f"sti{l}")
                 for l, d in ((1, cfg.H1), (2, cfg.H2))}
        st_out = {l: dram.tile([1, 2 * d], F32, tag=f"sto{l}", name=f"sto{l}",
                               addr_space="Shared")
                  for l, d in ((1, cfg.H1), (2, cfg.H2))}
        ss_dr = {l: dram.tile([1, 2 * d], F32, tag=f"ssd{l}", name=f"ssd{l}")
                 for l, d in ((1, cfg.H1), (2, cfg.H2))}
        z_sp = dram.tile([cfg.SH, cfg.H2], BF16, tag="zsp", name="zsp")

        cpool = top.enter_context(tc.tile_pool(name="const", bufs=1))
        ones_t = cpool.tile([P, 1], BF16, tag="ones")
        nc.vector.memset(ones_t[:], 1.0)
        ident = cpool.tile([P, P], BF16, tag="ident")
        make_identity(nc, ident[:])
        idx_lo_t = cpool.tile([P, TOT_LO * 8], I16, tag="idxlo")
        nc.sync.dma_start(idx_lo_t[:], idx_lo_d[:])
        idx_hi_t = cpool.tile([P, TOT_HI * 8], I16, tag="idxhi")
        nc.sync.dma_start(idx_hi_t[:], idx_hi_d[:])

        layers = [
            (1, cfg.IN, cfg.H1, x_full, x_own, True),
            (2, cfg.H1, cfg.H2, h_full[1], h_own[1], True),
            (3, cfg.H2, cfg.OUT, h_full[2], h_own[2], False),
        ]

        for (li, DI, DO, tab, own, bn) in layers:
            KI = DI // P
            n_oc = math.ceil(DO / 512)
            keep_z = bn and li == 1          # L1 z stays in SBUF
            GRP = 1 if bn else 3
            with ExitStack() as ls:
                sb = ls.enter_context(tc.tile_pool(name=f"sb{li}", bufs=2))
                sbz = ls.enter_context(tc.tile_pool(name=f"sbz{li}", bufs=4))
                sb1 = ls.enter_context(tc.tile_pool(name=f"sbw{li}", bufs=1))
                psA = ls.enter_context(tc.tile_pool(
                    name=f"psA{li}", bufs=(2 if bn else 1), space="PSUM"))
                psB = ls.enter_context(tc.tile_pool(name=f"psB{li}", bufs=1,
                                                    space="PSUM"))
                if bn:
                    psS = ls.enter_context(tc.tile_pool(name=f"psS{li}", bufs=1,
                                                        space="PSUM"))
                    sum_ps = psS.tile([1, DO], F32, tag="sum", space="PSUM")
                    sq_ps = psS.tile([1, DO], F32, tag="sq", space="PSUM")

                w_t = sb1.tile([P, KI * DO], BF16, tag="w")
                for k in range(KI):
                    nc.sync.dma_start(w_t[:, k * DO:(k + 1) * DO],
                                      w_d[li][k * P:(k + 1) * P, :])
                if not bn:
                    b3_t = sb1.tile([P, cfg.OUT], F32, tag="b3")
                    nc.sync.dma_start(b3_t[:], b3b_d[:])

                z_keep = []
                for b in range(cfg.BPC):
                    TLO, THI, NT = tlos[b], this_[b], nts[b]
                    gl = sb.tile([P, MAXLO, DI], BF16, tag="gl")
                    nc.gpsimd.dma_gather(
                        gl[:, :TLO, :], tab[0:cfg.LO_LIM, :],
                        idx_lo_t[:, off_lo[b] * 8:(off_lo[b] + TLO) * 8],
                        TLO * P, TLO * P, DI,
                        single_packet=(TLO * P <= 1024))
                    gh = sb.tile([P, MAXHI, DI], BF16, tag="gh")
                    nc.gpsimd.dma_gather(
                        gh[:, :THI, :], tab[cfg.LO_LIM:cfg.NPAD, :],
                        idx_hi_t[:, off_hi[b] * 8:(off_hi[b] + THI) * 8],
                        THI * P, THI * P, DI,
                        single_packet=(THI * P <= 1024))
                    own_t = sb.tile([P, DI], BF16, tag="own")
                    nc.scalar.dma_start(own_t[:], own[b * P:(b + 1) * P, :])
                    s_t = sb.tile([P, MAXNT * P], BF16, tag="s")
                    nc.sync.dma_start(
                        s_t[:, :NT * P], s_tab[:, off_s[b] * P:off_s[b + 1] * P])

                    agg = psA.tile([P, DI], F32, tag="agg", space="PSUM")
                    for t in range(NT):
                        if t < TLO:
                            rhs = gl[:, t, :]
                        elif t < TLO + THI:
                            rhs = gh[:, t - TLO, :]
                        else:
                            rhs = own_t[:]
                        for f0 in range(0, DI, 512):
                            fw = min(512, DI - f0)
                            nc.tensor.matmul(
                                agg[:, f0:f0 + fw],
                                lhsT=s_t[:, t * P:(t + 1) * P],
                                rhs=rhs[:, f0:f0 + fw],
                                start=(t == 0), stop=(t == NT - 1))
                    T_t = sb.tile([P, DI], BF16, tag="T")
                    nc.vector.tensor_copy(T_t[:], agg[:])

                    TT_t = sb.tile([P, DI], BF16, tag="TT")
                    for k in range(KI):
                        tp = psB.tile([P, P], BF16, tag="tp",
                                      bufs=(1 if bn else 2), space="PSUM")
                        nc.tensor.transpose(tp[:], T_t[:, k * P:(k + 1) * P],
                                            ident[:])
                        nc.vector.tensor_copy(TT_t[:, k * P:(k + 1) * P], tp[:])

                    if bn:
                        z_t = sbz.tile([P, DO], BF16, tag="z",
                                       bufs=(cfg.BPC + 1 if keep_z else 4))
                    else:
                        o_t = sb.tile([P, cfg.OUT], BF16, tag="o")
                    # main matmul; on L3 group ocs by 3 to reuse TT loads
                    for og in range(0, n_oc, GRP):
                        ocs = range(og, min(og + GRP, n_oc))
                        zps = {}
                        for oc in ocs:
                            ow = min(512, DO - oc * 512)
                            zps[oc] = psB.tile([P, ow], F32,
                                               tag=f"zp{oc % GRP}",
                                               name=f"zp{oc % GRP}",
                                               space="PSUM")
                        for k in range(KI):
                            for oc in ocs:
                                o0 = oc * 512
                                ow = min(512, DO - o0)
                                nc.tensor.matmul(
                                    zps[oc][:],
                                    lhsT=TT_t[:, k * P:(k + 1) * P],
                                    rhs=w_t[:, k * DO + o0:k * DO + o0 + ow],
                                    start=(k == 0), stop=(k == KI - 1))
                        for oc in ocs:
                            o0 = oc * 512
                            ow = min(512, DO - o0)
                            if bn:
                                nc.vector.tensor_copy(z_t[:, o0:o0 + ow],
                                                      zps[oc][:])
                            else:
                                nc.vector.tensor_add(o_t[:, o0:o0 + ow],
                                                     zps[oc][:],
                                                     b3_t[:, o0:o0 + ow])
                    if bn:
                        zsq = sb.tile([P, DO], BF16, tag="zsq")
                        nc.vector.tensor_tensor(
                            out=zsq[:], in0=z_t[:], in1=z_t[:],
                            op=mybir.AluOpType.mult)
                        for o0 in range(0, DO, 512):
                            ow = min(512, DO - o0)
                            nc.tensor.matmul(
                                sum_ps[:, o0:o0 + ow], lhsT=ones_t[:],
                                rhs=z_t[:, o0:o0 + ow],
                                start=(b == 0), stop=(b == cfg.BPC - 1))
                            nc.tensor.matmul(
                                sq_ps[:, o0:o0 + ow], lhsT=ones_t[:],
                                rhs=zsq[:, o0:o0 + ow],
                                start=(b == 0), stop=(b == cfg.BPC - 1))
                        if keep_z:
                            z_keep.append(z_t)
                        else:
                            nc.scalar.dma_start(
                                z_sp[b * P:(b + 1) * P, :DO], z_t[:])
                    else:
                        nc.scalar.dma_start(out_d[b * P:(b + 1) * P, :], o_t[:])

                if bn:
                    st_l = sb1.tile([1, 2 * DO], F32, tag="stl")
                    nc.vector.tensor_copy(st_l[:, 0:DO], sum_ps[:])
                    nc.vector.tensor_copy(st_l[:, DO:2 * DO], sq_ps[:])
                    nc.sync.dma_start(st_in[li][:, :], st_l[:])
                    nc.gpsimd.collective_compute(
                        "AllReduce", mybir.AluOpType.add,
                        replica_groups=rg,
                        ins=[st_in[li].opt()],
                        outs=[st_out[li].opt()])
                    st_g = sb1.tile([1, 2 * DO], F32, tag="stg")
                    nc.sync.dma_start(st_g[:], st_out[li][:, :])
                    g_t = sb1.tile([1, DO], F32, tag="g")
                    nc.sync.dma_start(g_t[:], g_d[li][:, :])
                    be_t = sb1.tile([1, DO], F32, tag="be")
                    nc.sync.dma_start(be_t[:], be_d[li][:, :])

                    inv_n = 1.0 / cfg.N_REAL
                    mean = sb1.tile([1, DO], F32, tag="mean")
                    nc.vector.tensor_scalar_mul(mean[:], st_g[:, 0:DO], inv_n)
                    msq = sb1.tile([1, DO], F32, tag="msq")
                    nc.vector.tensor_scalar_mul(msq[:], st_g[:, DO:2 * DO], inv_n)
                    var = sb1.tile([1, DO], F32, tag="var")
                    nc.vector.tensor_tensor(out=var[:], in0=mean[:], in1=mean[:],
                                            op=mybir.AluOpType.mult)
                    nc.vector.tensor_sub(var[:], msq[:], var[:])
                    nc.vector.tensor_scalar_add(var[:], var[:], EPS)
                    rec = sb1.tile([1, DO], F32, tag="rec")
                    nc.vector.reciprocal(rec[:], var[:])
                    zb = sb1.tile([1, 1], F32, tag="zb")
                    nc.vector.memset(zb[:], 0.0)
                    rstd = sb1.tile([1, DO], F32, tag="rstd")
                    nc.scalar.activation(rstd[:], rec[:],
                                         mybir.ActivationFunctionType.Sqrt,
                                         bias=zb[:])
                    ss_l = sb1.tile([1, 2 * DO], F32, tag="ssl")
                    nc.vector.tensor_tensor(out=ss_l[:, 0:DO], in0=g_t[:],
                                            in1=rstd[:], op=mybir.AluOpType.mult)
                    tmp = sb1.tile([1, DO], F32, tag="tmp")
                    nc.vector.tensor_tensor(out=tmp[:], in0=mean[:],
                                            in1=ss_l[:, 0:DO],
                                            op=mybir.AluOpType.mult)
                    nc.vector.tensor_sub(ss_l[:, DO:2 * DO], be_t[:], tmp[:])
                    nc.sync.dma_start(ss_dr[li][:, :], ss_l[:])
                    ssb_f = sb1.tile([P, 2 * DO], F32, tag="ssbf")
                    nc.sync.dma_start(
                        ssb_f[:], ss_dr[li][0:1, :].to_broadcast([P, 2 * DO]))
                    ssb = sb1.tile([P, 2 * DO], BF16, tag="ssb")
                    nc.vector.tensor_copy(ssb[:], ssb_f[:])

                    for b in range(cfg.BPC):
                        if keep_z:
                            zr = z_keep[b]
                        else:
                            zr = sbz.tile([P, DO], BF16, tag="zr")
                            nc.sync.dma_start(zr[:],
                                              z_sp[b * P:(b + 1) * P, :DO])
                        h_t = sbz.tile([P, DO], BF16, tag="h")
                        nc.vector.tensor_tensor(out=h_t[:], in0=zr[:],
                                                in1=ssb[:, 0:DO],
                                                op=mybir.AluOpType.mult)
                        nc.vector.tensor_tensor(out=h_t[:], in0=h_t[:],
                                                in1=ssb[:, DO:2 * DO],
                                                op=mybir.AluOpType.add)
                        nc.vector.tensor_relu(h_t[:], h_t[:])
                        nc.scalar.dma_start(
                            h_own[li][b * P:(b + 1) * P, :], h_t[:])
                        if (b + 1) % cfg.CG == 0:
                            g = b // cfg.CG
                            nc.gpsimd.collective_compute(
                                "AllGather", mybir.AluOpType.bypass,
                                replica_groups=rg,
                                ins=[h_own[li][g * CROWS:(g + 1) * CROWS, :].opt()],
                                outs=[h_full[li][g * cfg.NC * CROWS:
                                                 (g + 1) * cfg.NC * CROWS,
                                                 :].opt()])

    nc.compile()
    return nc


# ---------------------------------------------------------------------------

_FULL_CFG = CFG(50000, (256, 512, 1024, 3000))
_prog_cache = {}


def _get_program(cfg, tlos, this_):
    key = (cfg.N_REAL, cfg.IN, tuple(tlos), tuple(this_))
    if key not in _prog_cache:
        _prog_cache[key] = build_program(cfg, tlos, this_)
    return _prog_cache[key]


def kernel(x, edge_index, edge_attr, y, W1, b1, g1, be1, W2, b2, g2, be2, W3, b3):
    cfg = _FULL_CFG
    in_maps, tlos, this_ = host_prep(
        x, edge_index, edge_attr, W1, b1, g1, be1, W2, b2, g2, be2, W3, b3, cfg)
    nc = _get_program(cfg, tlos, this_)

    from concourse.bass_utils import run_bass_kernel_spmd
    res = run_bass_kernel_spmd(nc, in_maps, core_ids=list(range(cfg.NC)))
    out = np.empty((cfg.N_REAL, cfg.OUT), np.float32)
    for c in range(cfg.NC):
        out[c * cfg.NPC:(c + 1) * cfg.NPC] = \
            np.asarray(res.results[c]["out"])[:cfg.NPC].astype(np.float32)
    return out
